# revision 26
# baseline (speedup 1.0000x reference)
"""Trainium2 Bass kernel for ClassicalSelfAttention.

Math (per batch b):
    q = (x @ w_q.T) @ R ; k = (x @ w_k.T) @ Ent ; v = x @ w_v.T
    per head h: out_h = softmax(q_h @ k_h.T / sqrt(64)) @ v_h
    out[b, s, h*64+d]

Sharding: 8 cores, core i handles batch b = i // 4 and the adjacent head
pair m = i % 4 (global heads 2m, 2m+1 -> output columns 128m..128m+128).
Weights are replicated (column/row-sliced per head pair on the host); no
inter-core communication.

Per-core device plan (S = 4096, E = 512, D = 64, 2 heads):
  - host passes x^T and all weights in fp16 (halves input DMA and removes
    the on-device fp32->fp16 conversion copies; rel error of the fp16
    pipeline vs fp32 reference is ~7e-4).
  - combined projection weights Wq_comb = w_q.T @ rot_cols (and w_k/ent)
    are computed on PE in fp16 (1 cyc/row; fp32r had a 4x penalty at
    free=128), so q/k projections are single matmuls per 128-chunk.
  - Q^T, K^T stacked [128 = 2 heads x 64, S] in SBUF; V' = [V | ones]
    per head ([S, 65]) so the softmax denominator falls out of the PV
    matmul's extra column.
  - scores^T[k, q] = (K^T tile).T @ Q^T per head; the PE stream cost is
    the output free size (512 q per instruction), so scores+PV together
    are PSUM-write-port bound at ~221us/core; everything else must hide
    under that.
  - exp is split across two engines (ScalarE alone would be ~254us):
      * ACT: nc.scalar.activation(Exp, scale=1/8) PSUM->SBUF fp16.
      * DVE: one tensor_scalar per tile group computing the Schraudolph
        bit-trick in the fp16 domain: int16(s * 1024*log2(e)/8 + B)
        bit-cast as fp16 IS approximately exp(s/8) (max rel err ~4%,
        which softmax normalization mostly cancels; measured end-to-end
        rel-l2 ~5e-3 at a 1/3 DVE share).
    Groups are assigned round-robin ACT:DVE = 2:1.
  - PV: out'^T[65, q] = V'.T @ exp_sT accumulated over the 32 k-chunks in
    one PSUM bank per head; row 64 is the denominator. PE-transpose to
    natural layout, scale by reciprocal denominator, per-q-block DMA out.
  - Projections are streamed into the first q-block's spare PE cycles so
    the exp stream starts early.
"""

import sys

if "/opt/trn_rl_repo" not in sys.path:
    sys.path.insert(0, "/opt/trn_rl_repo")

import numpy as np

import concourse.bass as bass  # noqa: F401  (engine namespaces live on nc)
import concourse.mybir as mybir
import concourse.tile as tile
from concourse import bacc
from concourse.bass_utils import run_bass_kernel_spmd
from concourse.masks import make_identity

F32 = mybir.dt.float32
F16 = mybir.dt.float16
I16 = mybir.dt.int16
EXPF = mybir.ActivationFunctionType.Exp
MULT = mybir.AluOpType.mult
ADD = mybir.AluOpType.add

E = 512
D = 64
PAIR = 128  # 2 heads x 64 dims per core
N_CORES = 8

# Schraudolph constants for exp(s/8) in fp16-bit domain:
#   fp16_bits(exp(s/8)) ~= s * (1024*log2(e)/8) + (15*1024 - C)
SCH_A = float(np.float32(1024.0 * np.log2(np.e) / 8.0))
SCH_B = float(np.float32(15360.0 - 60.0))


def build_attention_nc(S=4096, lag=2, dve_pattern=(0, 0, 1, 0, 1), nslot=2,
                       sc_bufs=3, burst=3):
    """Build the single-core Bass program (SPMD: every core runs this).

    dve_pattern: cyclic pattern over exp slot-groups; 1 -> DVE Schraudolph,
    0 -> ACT exp.  nslot: score tiles per PSUM group.  sc_bufs: rotating
    PSUM group buffers.  burst: PV emission batching in groups (longer
    same-kind PE runs keep weight loads hidden and the p-state high).
    """
    EC = E // 128  # e-chunks (contraction over E)
    ST = S // 128  # s-tiles == k-tiles
    QB = S // 512  # query blocks (also projection blocks)
    NSLOT = nslot  # score slots (one head x k-tile each) per PSUM tile
    LAGS = NSLOT * lag  # PV lag in slots

    nc = bacc.Bacc("TRN2", target_bir_lowering=False, debug=False)

    xT_d = nc.dram_tensor("xT", [E, S], F16, kind="ExternalInput")
    wqc_d = nc.dram_tensor("wqc_cols", [E, PAIR], F16, kind="ExternalInput")
    wkc_d = nc.dram_tensor("wkc_cols", [E, PAIR], F16, kind="ExternalInput")
    wvT_d = nc.dram_tensor("wvT_cols", [E, PAIR], F16, kind="ExternalInput")
    out_d = nc.dram_tensor("out", [S, PAIR], F32, kind="ExternalOutput")

    with tile.TileContext(nc) as tc:
        with tc.tile_pool(name="persist", bufs=1) as PST:
            xT_sb = PST.tile([128, EC, S], F16)
            # per-block projection outputs so the main loop can start as
            # soon as block 0 is ready; remaining blocks interleave into
            # the loop's spare PE cycles.
            kTb = [PST.tile([128, 512], F16, name=f"kT_{b}") for b in range(QB)]
            qTb = [PST.tile([128, 512], F16, name=f"qT_{b}") for b in range(QB)]
            # V' per k-chunk: [V_h0 (64) | 1 | V_h1 (64) | 1]
            vb = [PST.tile([128, 4, 130], F16, name=f"v_{b}") for b in range(QB)]
            out_sb = PST.tile([128, ST, PAIR], F32)
            ident = PST.tile([128, 128], F32)
            wqc_sb = PST.tile([128, EC, PAIR], F16)
            wkc_sb = PST.tile([128, EC, PAIR], F16)
            wvT_sb = PST.tile([128, EC, PAIR], F16)
            make_identity(nc, ident[:])
            # memset can't target fp16; stage in fp32 and convert-copy
            ones_f32 = PST.tile([128, 4], F32)
            nc.vector.memset(ones_f32[:], 1.0)

            # ------------- weight + x DMAs (weights host-combined) -------
            for c in range(EC):
                sl = slice(128 * c, 128 * (c + 1))
                nc.sync.dma_start(out=wkc_sb[:, c, :], in_=wkc_d[sl, :])
                nc.sync.dma_start(out=wqc_sb[:, c, :], in_=wqc_d[sl, :])
                nc.sync.dma_start(out=wvT_sb[:, c, :], in_=wvT_d[sl, :])
            for b in range(QB):
                for c in range(EC):
                    sl = slice(128 * c, 128 * (c + 1))
                    for p4 in range(4):
                        bs = slice(512 * b + 128 * p4, 512 * b + 128 * (p4 + 1))
                        nc.sync.dma_start(out=xT_sb[:, c, bs], in_=xT_d[sl, bs])

            # ---------------- attention main loop -----------------------
            with (
                tc.tile_pool(name="sc_ps", bufs=sc_bufs, space="PSUM") as SC,
                tc.tile_pool(name="pv_ps", bufs=2, space="PSUM") as PVP,
                tc.tile_pool(name="exp_sb", bufs=11) as EX,
                tc.tile_pool(name="nrm_sb", bufs=2) as NRM,
            ):
                # projection emitters; psum borrowed from the score pool so
                # they can interleave with the loop without extra banks
                def emit_kqT(b, wc, dst, kind):
                    ps = SC.tile([128, NSLOT, 512], F32, tag="sc", name=f"{kind}ps_{b}")
                    bs = slice(512 * b, 512 * (b + 1))
                    for c in range(EC):
                        nc.tensor.matmul(
                            ps[:, 0, :],
                            lhsT=wc[:, c, :],
                            rhs=xT_sb[:, c, bs],
                            start=(c == 0),
                            stop=(c == EC - 1),
                        )
                    nc.vector.tensor_copy(dst[:], ps[:, 0, :])

                def emit_v(b):
                    ps = SC.tile([128, NSLOT, 512], F32, tag="sc", name=f"vps_{b}")
                    view = ps[:, 0, :].rearrange("p (j n) -> p j n", j=4)
                    for jj in range(4):
                        j = 4 * b + jj
                        for c in range(EC):
                            nc.tensor.matmul(
                                view[:, jj, :],
                                lhsT=xT_sb[:, c, 128 * j : 128 * (j + 1)],
                                rhs=wvT_sb[:, c, :],
                                start=(c == 0),
                                stop=(c == EC - 1),
                            )
                    nc.vector.tensor_copy(vb[b][:, :, 0:64], view[:, :, 0:64])
                    nc.vector.tensor_copy(vb[b][:, :, 65:129], view[:, :, 64:128])
                    nc.vector.tensor_copy(vb[b][:, :, 64:65], ones_f32[:])
                    nc.vector.tensor_copy(vb[b][:, :, 129:130], ones_f32[:])

                # minimal pre-loop: block 0 (+1 block of lookahead)
                emit_kqT(0, wkc_sb, kTb[0], "k")
                emit_kqT(0, wqc_sb, qTb[0], "q")
                emit_v(0)
                if QB > 1:
                    emit_kqT(1, wkc_sb, kTb[1], "k")
                    emit_v(1)

                # remaining blocks fill qb0's spare PE cycles: one unit
                # every 2 k-tiles, always ahead of first use (kt = 4b)
                proj_sched = {}
                for b in range(2, QB):
                    proj_sched.setdefault(4 * b - 2, []).append(("k", b))
                    proj_sched.setdefault(4 * b - 1, []).append(("v", b))

                # cyclic exp-engine assignment per slot-group
                group_count = [0]

                def exp_emit(et, sc, n_slots):
                    g = group_count[0]
                    group_count[0] += 1
                    use_dve = dve_pattern[g % len(dve_pattern)]
                    if use_dve:
                        nc.vector.tensor_scalar(
                            et[:, :n_slots, :].bitcast(I16),
                            sc[:, :n_slots, :],
                            SCH_A,
                            SCH_B,
                            MULT,
                            ADD,
                        )
                    else:
                        nc.scalar.activation(
                            et[:, :n_slots, :], sc[:, :n_slots, :], EXPF, scale=0.125
                        )

                # normalize/output for a finished q-block, split in two
                # phases so they can be deferred into the next q-block's
                # score stream without ever stalling the PE.
                def finish_a(fin):
                    qbp, pv, tr, pvS = fin[:4]
                    for h in range(2):
                        pvS.append(
                            NRM.tile([65, 512], F32, tag="pvS", name=f"pvS_{qbp}_{h}")
                        )
                        nc.vector.tensor_copy(pvS[h][:], pv[h][0:65, :])

                def finish_b(fin):
                    qbp, pv, tr, pvS = fin[:4]
                    recs = []
                    for h in range(2):
                        for c4 in range(4):
                            nc.tensor.transpose(
                                tr[h][:, c4, :],
                                pvS[h][:, 128 * c4 : 128 * (c4 + 1)],
                                ident[0:65, 0:65],
                            )
                        rec = NRM.tile([128, 4], F32, tag="rec", name=f"rec_{qbp}_{h}")
                        nc.vector.reciprocal(rec[:], tr[h][:, :, 64])
                        recs.append(rec)
                    for c4 in range(4):
                        j = 4 * qbp + c4
                        for h in range(2):
                            nc.vector.tensor_scalar_mul(
                                out_sb[:, j, 64 * h : 64 * (h + 1)],
                                tr[h][:, c4, 0:64],
                                recs[h][:, c4 : c4 + 1],
                            )
                        nc.sync.dma_start(
                            out=out_d[128 * j : 128 * (j + 1), :],
                            in_=out_sb[:, j, :],
                        )

                pending = None
                for qb in range(QB):
                    pv = [
                        PVP.tile([128, 512], F32, tag="pv", name=f"pv_h0_{qb}"),
                        PVP.tile([128, 512], F32, tag="pv", name=f"pv_h1_{qb}"),
                    ]
                    tr = [
                        PVP.tile([128, 4, 65], F32, tag="pv", name=f"tr_{qb}_{h}")
                        for h in range(2)
                    ]
                    slot_et = [None] * (2 * ST)  # slot -> (exp tile, pos)
                    state = {"sc": None, "et": None, "acted": -1, "pv_next": 0}

                    def emit_pv(s, pv=pv, slot_et=slot_et):
                        kt, h = divmod(s, 2)
                        et, pos = slot_et[s]
                        nc.tensor.matmul(
                            pv[h][0:65, :],
                            lhsT=vb[kt // 4][:, kt % 4, 65 * h : 65 * h + 65],
                            rhs=et[:, pos, :],
                            start=(kt == 0),
                            stop=(kt == ST - 1),
                        )

                    def drain_pv(upto, state=state, emit=emit_pv):
                        while state["pv_next"] <= upto:
                            emit(state["pv_next"])
                            state["pv_next"] += 1

                    for kt in range(ST):
                        if pending is not None:
                            if kt == 2:
                                pending[4]()  # tail PV drain of previous qb
                            elif kt == 3:
                                finish_a(pending)
                            elif kt == 4:
                                finish_b(pending)
                                pending = None
                        if qb == 0:
                            for kind, b in proj_sched.get(kt, ()):
                                if kind == "k":
                                    emit_kqT(b, wkc_sb, kTb[b], "k")
                                else:
                                    emit_v(b)
                        if kt == 16 and qb + 1 < QB:
                            emit_kqT(qb + 1, wqc_sb, qTb[qb + 1], "q")
                        for h in range(2):
                            s = 2 * kt + h
                            pos = s % NSLOT
                            if pos == 0:
                                state["sc"] = SC.tile(
                                    [128, NSLOT, 512], F32, tag="sc", name=f"sc_{qb}_{s}"
                                )
                                state["et"] = EX.tile(
                                    [128, NSLOT, 512], F16, tag="et", name=f"et_{qb}_{s}"
                                )
                            nc.tensor.matmul(
                                state["sc"][:, pos, :],
                                lhsT=kTb[kt // 4][64 * h : 64 * (h + 1), 128 * (kt % 4) : 128 * (kt % 4 + 1)],
                                rhs=qTb[qb][64 * h : 64 * (h + 1), :],
                                start=True,
                                stop=True,
                            )
                            slot_et[s] = (state["et"], pos)
                            if pos == NSLOT - 1:
                                exp_emit(state["et"], state["sc"], NSLOT)
                                state["acted"] = s
                                state["groups"] = state.get("groups", 0) + 1
                                # the first drain of a deferred-finish block
                                # must come after finish_b releases the pv
                                # buffers (else PE deadlocks on its own
                                # later transposes)
                                min_g = burst if qb == 0 else 2 * burst
                                if (
                                    state["groups"] % burst == 0
                                    and state["groups"] >= min_g
                                ):
                                    drain_pv(state["acted"] - LAGS)
                    # flush partial tile; tail PVs are deferred into the
                    # next q-block's score stream (their exps need ~1us)
                    last = 2 * ST - 1
                    if state["acted"] < last:
                        pos = last % NSLOT
                        exp_emit(state["et"], state["sc"], pos + 1)
                    drain_pv(last - 2 * NSLOT)
                    pending = (qb, pv, tr, [], lambda d=drain_pv: d(last), [])

                pending[4]()
                finish_a(pending)
                finish_b(pending)

    nc.compile()
    return nc


_NC_CACHE = {}

BUILD_OPTS = {"lag": 2, "dve_pattern": (0, 0, 1, 0, 1), "nslot": 2,
              "sc_bufs": 3, "burst": 3}


def _get_nc(S=4096):
    key = (S,) + tuple(sorted((k, tuple(v) if isinstance(v, tuple) else v)
                              for k, v in BUILD_OPTS.items()))
    if key not in _NC_CACHE:
        _NC_CACHE[key] = build_attention_nc(S=S, **BUILD_OPTS)
    return _NC_CACHE[key]


def _make_in_maps(rotation_params, entangle_params, inputs, w_q, w_k, w_v):
    B, S, E_ = inputs.shape
    assert E_ == E and B * 4 == N_CORES
    f16 = lambda a: np.ascontiguousarray(np.asarray(a, dtype=np.float16))
    xTs = [f16(np.asarray(inputs[b]).T) for b in range(B)]
    w_q = np.asarray(w_q, dtype=np.float32)
    w_k = np.asarray(w_k, dtype=np.float32)
    rotation_params = np.asarray(rotation_params, dtype=np.float32)
    entangle_params = np.asarray(entangle_params, dtype=np.float32)
    w_v = np.asarray(w_v)
    # combined projection weights (host, fp32 accumulate -> fp16)
    wqc = w_q.T @ rotation_params
    wkc = w_k.T @ entangle_params
    in_maps = []
    for core in range(N_CORES):
        b, m = divmod(core, 4)
        cols = slice(PAIR * m, PAIR * (m + 1))
        in_maps.append(
            {
                "xT": xTs[b],
                "wqc_cols": f16(wqc[:, cols]),
                "wkc_cols": f16(wkc[:, cols]),
                "wvT_cols": f16(w_v[cols, :].T),
            }
        )
    return in_maps


def run(rotation_params, entangle_params, inputs, w_q, w_k, w_v, trace=False):
    """Run on the 8 NeuronCores; returns (output, BassKernelResults)."""
    inputs = np.asarray(inputs)
    B, S, E_ = inputs.shape
    nc = _get_nc(S)
    in_maps = _make_in_maps(rotation_params, entangle_params, inputs, w_q, w_k, w_v)
    res = run_bass_kernel_spmd(nc, in_maps, list(range(N_CORES)), trace=trace)
    out = np.empty((B, S, E_), dtype=np.float32)
    for core in range(N_CORES):
        b, m = divmod(core, 4)
        out[b, :, PAIR * m : PAIR * (m + 1)] = res.results[core]["out"]
    return out, res


def kernel(rotation_params, entangle_params, inputs, w_q, w_k, w_v):
    out, _ = run(rotation_params, entangle_params, inputs, w_q, w_k, w_v)
    return out


# revision 27
# speedup vs baseline: 1.0855x; 1.0855x over previous
"""Trainium2 Bass kernel for ClassicalSelfAttention.

Math (per batch b):
    q = (x @ w_q.T) @ R ; k = (x @ w_k.T) @ Ent ; v = x @ w_v.T
    per head h: out_h = softmax(q_h @ k_h.T / sqrt(64)) @ v_h
    out[b, s, h*64+d]

Sharding: 8 cores, core i handles batch b = i // 4 and the adjacent head
pair m = i % 4 (global heads 2m, 2m+1 -> output columns 128m..128m+128).
Weights are replicated (column/row-sliced per head pair on the host); no
inter-core communication.

Per-core device plan (S = 4096, E = 512, D = 64, 2 heads):
  - host passes x^T and all weights in fp16 (halves input DMA and removes
    the on-device fp32->fp16 conversion copies; rel error of the fp16
    pipeline vs fp32 reference is ~7e-4).
  - combined projection weights Wq_comb = w_q.T @ rot_cols (and w_k/ent)
    are computed on PE in fp16 (1 cyc/row; fp32r had a 4x penalty at
    free=128), so q/k projections are single matmuls per 128-chunk.
  - Q^T, K^T stacked [128 = 2 heads x 64, S] in SBUF; V' = [V | ones]
    per head ([S, 65]) so the softmax denominator falls out of the PV
    matmul's extra column.
  - scores^T[k, q] = (K^T tile).T @ Q^T per head; the PE stream cost is
    the output free size (512 q per instruction), so scores+PV together
    are PSUM-write-port bound at ~221us/core; everything else must hide
    under that.
  - exp is split across two engines (ScalarE alone would be ~254us):
      * ACT: nc.scalar.activation(Exp, scale=1/8) PSUM->SBUF fp16.
      * DVE: one tensor_scalar per tile group computing the Schraudolph
        bit-trick in the fp16 domain: int16(s * 1024*log2(e)/8 + B)
        bit-cast as fp16 IS approximately exp(s/8) (max rel err ~4%,
        which softmax normalization mostly cancels; measured end-to-end
        rel-l2 ~5e-3 at a 1/3 DVE share).
    Groups are assigned round-robin ACT:DVE = 2:1.
  - PV: out'^T[65, q] = V'.T @ exp_sT accumulated over the 32 k-chunks in
    one PSUM bank per head; row 64 is the denominator. PE-transpose to
    natural layout, scale by reciprocal denominator, per-q-block DMA out.
  - Projections are streamed into the first q-block's spare PE cycles so
    the exp stream starts early.
"""

import sys

if "/opt/trn_rl_repo" not in sys.path:
    sys.path.insert(0, "/opt/trn_rl_repo")

import numpy as np

import concourse.bass as bass  # noqa: F401  (engine namespaces live on nc)
import concourse.mybir as mybir
import concourse.tile as tile
from concourse import bacc
from concourse.bass_utils import run_bass_kernel_spmd
from concourse.masks import make_identity

F32 = mybir.dt.float32
F16 = mybir.dt.float16
I16 = mybir.dt.int16
EXPF = mybir.ActivationFunctionType.Exp
MULT = mybir.AluOpType.mult
ADD = mybir.AluOpType.add

E = 512
D = 64
PAIR = 128  # 2 heads x 64 dims per core
N_CORES = 8

# Schraudolph constants for exp(s/8) in fp16-bit domain:
#   fp16_bits(exp(s/8)) ~= s * (1024*log2(e)/8) + (15*1024 - C)
SCH_A = float(np.float32(1024.0 * np.log2(np.e) / 8.0))
SCH_B = float(np.float32(15360.0 - 60.0))


def build_attention_nc(S=4096, lag=2, dve_pattern=(0, 0, 1, 0, 1), nslot=2,
                       sc_bufs=3, burst=3):
    """Build the single-core Bass program (SPMD: every core runs this).

    dve_pattern: cyclic pattern over exp slot-groups; 1 -> DVE Schraudolph,
    0 -> ACT exp.  nslot: score tiles per PSUM group.  sc_bufs: rotating
    PSUM group buffers.  burst: PV emission batching in groups (longer
    same-kind PE runs keep weight loads hidden and the p-state high).
    """
    EC = E // 128  # e-chunks (contraction over E)
    ST = S // 128  # s-tiles == k-tiles
    QB = S // 512  # query blocks (also projection blocks)
    NSLOT = nslot  # score slots (one head x k-tile each) per PSUM tile
    LAGS = NSLOT * lag  # PV lag in slots

    nc = bacc.Bacc("TRN2", target_bir_lowering=False, debug=False)

    xT_d = nc.dram_tensor("xT", [E, S], F16, kind="ExternalInput")
    wqc_d = nc.dram_tensor("wqc_cols", [E, PAIR], F16, kind="ExternalInput")
    wkc_d = nc.dram_tensor("wkc_cols", [E, PAIR], F16, kind="ExternalInput")
    wvT_d = nc.dram_tensor("wvT_cols", [E, PAIR], F16, kind="ExternalInput")
    out_d = nc.dram_tensor("out", [S, PAIR], F32, kind="ExternalOutput")

    with tile.TileContext(nc) as tc:
        with tc.tile_pool(name="persist", bufs=1) as PST:
            xT_sb = PST.tile([128, EC, S], F16)
            # per-block projection outputs so the main loop can start as
            # soon as block 0 is ready; remaining blocks interleave into
            # the loop's spare PE cycles.
            kTb = [PST.tile([128, 512], F16, name=f"kT_{b}") for b in range(QB)]
            qTb = [PST.tile([128, 512], F16, name=f"qT_{b}") for b in range(QB)]
            # V' per k-chunk: [V_h0 (64) | 1 | V_h1 (64) | 1]
            vb = [PST.tile([128, 4, 130], F16, name=f"v_{b}") for b in range(QB)]
            out_sb = PST.tile([128, ST, PAIR], F32)
            ident = PST.tile([128, 128], F32)
            wqc_sb = PST.tile([128, EC, PAIR], F16)
            wkc_sb = PST.tile([128, EC, PAIR], F16)
            wvT_sb = PST.tile([128, EC, PAIR], F16)
            make_identity(nc, ident[:])
            # memset can't target fp16; stage in fp32 and convert-copy
            ones_f32 = PST.tile([128, 4], F32)
            nc.vector.memset(ones_f32[:], 1.0)

            # ------------- weight + x DMAs (weights host-combined) -------
            for c in range(EC):
                sl = slice(128 * c, 128 * (c + 1))
                nc.sync.dma_start(out=wkc_sb[:, c, :], in_=wkc_d[sl, :])
                nc.sync.dma_start(out=wqc_sb[:, c, :], in_=wqc_d[sl, :])
                nc.sync.dma_start(out=wvT_sb[:, c, :], in_=wvT_d[sl, :])
            for b in range(QB):
                bs = slice(512 * b, 512 * (b + 1))
                nsub = 4 if b < 2 else 2
                for c in range(EC):
                    for p4 in range(nsub):
                        rows = 128 // nsub
                        sl = slice(128 * c + rows * p4, 128 * c + rows * (p4 + 1))
                        nc.sync.dma_start(
                            out=xT_sb[rows * p4 : rows * (p4 + 1), c, bs],
                            in_=xT_d[sl, bs],
                        )

            # ---------------- attention main loop -----------------------
            with (
                tc.tile_pool(name="sc_ps", bufs=sc_bufs, space="PSUM") as SC,
                tc.tile_pool(name="pv_ps", bufs=2, space="PSUM") as PVP,
                tc.tile_pool(name="exp_sb", bufs=11) as EX,
                tc.tile_pool(name="nrm_sb", bufs=2) as NRM,
            ):
                # projection emitters; psum borrowed from the score pool so
                # they can interleave with the loop without extra banks
                def emit_kqT(b, wc, dst, kind):
                    ps = SC.tile([128, NSLOT, 512], F32, tag="sc", name=f"{kind}ps_{b}")
                    bs = slice(512 * b, 512 * (b + 1))
                    for c in range(EC):
                        nc.tensor.matmul(
                            ps[:, 0, :],
                            lhsT=wc[:, c, :],
                            rhs=xT_sb[:, c, bs],
                            start=(c == 0),
                            stop=(c == EC - 1),
                        )
                    nc.vector.tensor_copy(dst[:], ps[:, 0, :])

                def emit_v(b):
                    ps = SC.tile([128, NSLOT, 512], F32, tag="sc", name=f"vps_{b}")
                    view = ps[:, 0, :].rearrange("p (j n) -> p j n", j=4)
                    for jj in range(4):
                        j = 4 * b + jj
                        for c in range(EC):
                            nc.tensor.matmul(
                                view[:, jj, :],
                                lhsT=xT_sb[:, c, 128 * j : 128 * (j + 1)],
                                rhs=wvT_sb[:, c, :],
                                start=(c == 0),
                                stop=(c == EC - 1),
                            )
                    nc.vector.tensor_copy(vb[b][:, :, 0:64], view[:, :, 0:64])
                    nc.vector.tensor_copy(vb[b][:, :, 65:129], view[:, :, 64:128])
                    nc.vector.tensor_copy(vb[b][:, :, 64:65], ones_f32[:])
                    nc.vector.tensor_copy(vb[b][:, :, 129:130], ones_f32[:])

                # minimal pre-loop: block 0 (+1 block of lookahead)
                emit_kqT(0, wkc_sb, kTb[0], "k")
                emit_kqT(0, wqc_sb, qTb[0], "q")
                emit_v(0)
                if QB > 1:
                    emit_kqT(1, wkc_sb, kTb[1], "k")
                    emit_v(1)

                # remaining blocks fill qb0's spare PE cycles: one unit
                # every 2 k-tiles, always ahead of first use (kt = 4b)
                proj_sched = {}
                for b in range(2, QB):
                    proj_sched.setdefault(4 * b - 2, []).append(("k", b))
                    proj_sched.setdefault(4 * b - 1, []).append(("v", b))

                # cyclic exp-engine assignment per slot-group
                group_count = [0]

                def exp_emit(et, sc, n_slots):
                    g = group_count[0]
                    group_count[0] += 1
                    use_dve = dve_pattern[g % len(dve_pattern)]
                    if use_dve:
                        nc.vector.tensor_scalar(
                            et[:, :n_slots, :].bitcast(I16),
                            sc[:, :n_slots, :],
                            SCH_A,
                            SCH_B,
                            MULT,
                            ADD,
                        )
                    else:
                        nc.scalar.activation(
                            et[:, :n_slots, :], sc[:, :n_slots, :], EXPF, scale=0.125
                        )

                # normalize/output for a finished q-block, split in two
                # phases so they can be deferred into the next q-block's
                # score stream without ever stalling the PE.
                def finish_a(fin):
                    qbp, pv, tr, pvS = fin[:4]
                    for h in range(2):
                        pvS.append(
                            NRM.tile([65, 512], F32, tag="pvS", name=f"pvS_{qbp}_{h}")
                        )
                        nc.vector.tensor_copy(pvS[h][:], pv[h][0:65, :])

                def finish_b(fin):
                    qbp, pv, tr, pvS = fin[:4]
                    recs = []
                    for h in range(2):
                        for c4 in range(4):
                            nc.tensor.transpose(
                                tr[h][:, c4, :],
                                pvS[h][:, 128 * c4 : 128 * (c4 + 1)],
                                ident[0:65, 0:65],
                            )
                        rec = NRM.tile([128, 4], F32, tag="rec", name=f"rec_{qbp}_{h}")
                        nc.vector.reciprocal(rec[:], tr[h][:, :, 64])
                        recs.append(rec)
                    for c4 in range(4):
                        j = 4 * qbp + c4
                        for h in range(2):
                            nc.vector.tensor_scalar_mul(
                                out_sb[:, j, 64 * h : 64 * (h + 1)],
                                tr[h][:, c4, 0:64],
                                recs[h][:, c4 : c4 + 1],
                            )
                        nc.sync.dma_start(
                            out=out_d[128 * j : 128 * (j + 1), :],
                            in_=out_sb[:, j, :],
                        )

                pending = None
                for qb in range(QB):
                    pv = [
                        PVP.tile([128, 512], F32, tag="pv", name=f"pv_h0_{qb}"),
                        PVP.tile([128, 512], F32, tag="pv", name=f"pv_h1_{qb}"),
                    ]
                    tr = [
                        PVP.tile([128, 4, 65], F32, tag="pv", name=f"tr_{qb}_{h}")
                        for h in range(2)
                    ]
                    slot_et = [None] * (2 * ST)  # slot -> (exp tile, pos)
                    state = {"sc": None, "et": None, "acted": -1, "pv_next": 0}

                    def emit_pv(s, pv=pv, slot_et=slot_et):
                        kt, h = divmod(s, 2)
                        et, pos = slot_et[s]
                        nc.tensor.matmul(
                            pv[h][0:65, :],
                            lhsT=vb[kt // 4][:, kt % 4, 65 * h : 65 * h + 65],
                            rhs=et[:, pos, :],
                            start=(kt == 0),
                            stop=(kt == ST - 1),
                        )

                    def drain_pv(upto, state=state, emit=emit_pv):
                        while state["pv_next"] <= upto:
                            emit(state["pv_next"])
                            state["pv_next"] += 1

                    for kt in range(ST):
                        if pending is not None:
                            if kt == 2:
                                pending[4]()  # tail PV drain of previous qb
                            elif kt == 3:
                                finish_a(pending)
                            elif kt == 4:
                                finish_b(pending)
                                pending = None
                        if qb == 0:
                            for kind, b in proj_sched.get(kt, ()):
                                if kind == "k":
                                    emit_kqT(b, wkc_sb, kTb[b], "k")
                                else:
                                    emit_v(b)
                        if kt == 16 and qb + 1 < QB:
                            emit_kqT(qb + 1, wqc_sb, qTb[qb + 1], "q")
                        for h in range(2):
                            s = 2 * kt + h
                            pos = s % NSLOT
                            if pos == 0:
                                state["sc"] = SC.tile(
                                    [128, NSLOT, 512], F32, tag="sc", name=f"sc_{qb}_{s}"
                                )
                                state["et"] = EX.tile(
                                    [128, NSLOT, 512], F16, tag="et", name=f"et_{qb}_{s}"
                                )
                            nc.tensor.matmul(
                                state["sc"][:, pos, :],
                                lhsT=kTb[kt // 4][64 * h : 64 * (h + 1), 128 * (kt % 4) : 128 * (kt % 4 + 1)],
                                rhs=qTb[qb][64 * h : 64 * (h + 1), :],
                                start=True,
                                stop=True,
                            )
                            slot_et[s] = (state["et"], pos)
                            if pos == NSLOT - 1:
                                exp_emit(state["et"], state["sc"], NSLOT)
                                state["acted"] = s
                                state["groups"] = state.get("groups", 0) + 1
                                # the first drain of a deferred-finish block
                                # must come after finish_b releases the pv
                                # buffers (else PE deadlocks on its own
                                # later transposes)
                                min_g = burst if qb == 0 else 2 * burst
                                if (
                                    state["groups"] % burst == 0
                                    and state["groups"] >= min_g
                                ):
                                    drain_pv(state["acted"] - LAGS)
                    # flush partial tile; tail PVs are deferred into the
                    # next q-block's score stream (their exps need ~1us)
                    last = 2 * ST - 1
                    if state["acted"] < last:
                        pos = last % NSLOT
                        exp_emit(state["et"], state["sc"], pos + 1)
                    drain_pv(last - 2 * NSLOT)
                    pending = (qb, pv, tr, [], lambda d=drain_pv: d(last), [])

                pending[4]()
                finish_a(pending)
                finish_b(pending)

    nc.compile()
    return nc


_NC_CACHE = {}

BUILD_OPTS = {"lag": 2, "dve_pattern": (0, 0, 1, 0, 1), "nslot": 2,
              "sc_bufs": 3, "burst": 3}


def _get_nc(S=4096):
    key = (S,) + tuple(sorted((k, tuple(v) if isinstance(v, tuple) else v)
                              for k, v in BUILD_OPTS.items()))
    if key not in _NC_CACHE:
        _NC_CACHE[key] = build_attention_nc(S=S, **BUILD_OPTS)
    return _NC_CACHE[key]


def _make_in_maps(rotation_params, entangle_params, inputs, w_q, w_k, w_v):
    B, S, E_ = inputs.shape
    assert E_ == E and B * 4 == N_CORES
    f16 = lambda a: np.ascontiguousarray(np.asarray(a, dtype=np.float16))
    xTs = [f16(np.asarray(inputs[b]).T) for b in range(B)]
    w_q = np.asarray(w_q, dtype=np.float32)
    w_k = np.asarray(w_k, dtype=np.float32)
    rotation_params = np.asarray(rotation_params, dtype=np.float32)
    entangle_params = np.asarray(entangle_params, dtype=np.float32)
    w_v = np.asarray(w_v)
    # combined projection weights (host, fp32 accumulate -> fp16)
    wqc = w_q.T @ rotation_params
    wkc = w_k.T @ entangle_params
    in_maps = []
    for core in range(N_CORES):
        b, m = divmod(core, 4)
        cols = slice(PAIR * m, PAIR * (m + 1))
        in_maps.append(
            {
                "xT": xTs[b],
                "wqc_cols": f16(wqc[:, cols]),
                "wkc_cols": f16(wkc[:, cols]),
                "wvT_cols": f16(w_v[cols, :].T),
            }
        )
    return in_maps


def run(rotation_params, entangle_params, inputs, w_q, w_k, w_v, trace=False):
    """Run on the 8 NeuronCores; returns (output, BassKernelResults)."""
    inputs = np.asarray(inputs)
    B, S, E_ = inputs.shape
    nc = _get_nc(S)
    in_maps = _make_in_maps(rotation_params, entangle_params, inputs, w_q, w_k, w_v)
    res = run_bass_kernel_spmd(nc, in_maps, list(range(N_CORES)), trace=trace)
    out = np.empty((B, S, E_), dtype=np.float32)
    for core in range(N_CORES):
        b, m = divmod(core, 4)
        out[b, :, PAIR * m : PAIR * (m + 1)] = res.results[core]["out"]
    return out, res


def kernel(rotation_params, entangle_params, inputs, w_q, w_k, w_v):
    out, _ = run(rotation_params, entangle_params, inputs, w_q, w_k, w_v)
    return out


# revision 28
# speedup vs baseline: 1.0930x; 1.0069x over previous
"""Trainium2 Bass kernel for ClassicalSelfAttention.

Math (per batch b):
    q = (x @ w_q.T) @ R ; k = (x @ w_k.T) @ Ent ; v = x @ w_v.T
    per head h: out_h = softmax(q_h @ k_h.T / sqrt(64)) @ v_h
    out[b, s, h*64+d]

Sharding: 8 cores, core i handles batch b = i // 4 and the adjacent head
pair m = i % 4 (global heads 2m, 2m+1 -> output columns 128m..128m+128).
Weights are replicated (column/row-sliced per head pair on the host); no
inter-core communication.

Per-core device plan (S = 4096, E = 512, D = 64, 2 heads):
  - host passes x^T and all weights in fp16 (halves input DMA and removes
    the on-device fp32->fp16 conversion copies; rel error of the fp16
    pipeline vs fp32 reference is ~7e-4).
  - combined projection weights Wq_comb = w_q.T @ rot_cols (and w_k/ent)
    are computed on PE in fp16 (1 cyc/row; fp32r had a 4x penalty at
    free=128), so q/k projections are single matmuls per 128-chunk.
  - Q^T, K^T stacked [128 = 2 heads x 64, S] in SBUF; V' = [V | ones]
    per head ([S, 65]) so the softmax denominator falls out of the PV
    matmul's extra column.
  - scores^T[k, q] = (K^T tile).T @ Q^T per head; the PE stream cost is
    the output free size (512 q per instruction), so scores+PV together
    are PSUM-write-port bound at ~221us/core; everything else must hide
    under that.
  - exp is split across two engines (ScalarE alone would be ~254us):
      * ACT: nc.scalar.activation(Exp, scale=1/8) PSUM->SBUF fp16.
      * DVE: one tensor_scalar per tile group computing the Schraudolph
        bit-trick in the fp16 domain: int16(s * 1024*log2(e)/8 + B)
        bit-cast as fp16 IS approximately exp(s/8) (max rel err ~4%,
        which softmax normalization mostly cancels; measured end-to-end
        rel-l2 ~5e-3 at a 1/3 DVE share).
    Groups are assigned round-robin ACT:DVE = 2:1.
  - PV: out'^T[65, q] = V'.T @ exp_sT accumulated over the 32 k-chunks in
    one PSUM bank per head; row 64 is the denominator. PE-transpose to
    natural layout, scale by reciprocal denominator, per-q-block DMA out.
  - Projections are streamed into the first q-block's spare PE cycles so
    the exp stream starts early.
"""

import sys

if "/opt/trn_rl_repo" not in sys.path:
    sys.path.insert(0, "/opt/trn_rl_repo")

import numpy as np

import concourse.bass as bass  # noqa: F401  (engine namespaces live on nc)
import concourse.mybir as mybir
import concourse.tile as tile
from concourse import bacc
from concourse.bass_utils import run_bass_kernel_spmd
from concourse.masks import make_identity

F32 = mybir.dt.float32
F16 = mybir.dt.float16
I16 = mybir.dt.int16
EXPF = mybir.ActivationFunctionType.Exp
MULT = mybir.AluOpType.mult
ADD = mybir.AluOpType.add

E = 512
D = 64
PAIR = 128  # 2 heads x 64 dims per core
N_CORES = 8

# Schraudolph constants for exp(s/8) in fp16-bit domain:
#   fp16_bits(exp(s/8)) ~= s * (1024*log2(e)/8) + (15*1024 - C)
SCH_A = float(np.float32(1024.0 * np.log2(np.e) / 8.0))
SCH_B = float(np.float32(15360.0 - 60.0))


def build_attention_nc(S=4096, lag=2, dve_pattern=(0, 0, 1, 0, 1), nslot=2,
                       sc_bufs=3, burst=3):
    """Build the single-core Bass program (SPMD: every core runs this).

    dve_pattern: cyclic pattern over exp slot-groups; 1 -> DVE Schraudolph,
    0 -> ACT exp.  nslot: score tiles per PSUM group.  sc_bufs: rotating
    PSUM group buffers.  burst: PV emission batching in groups (longer
    same-kind PE runs keep weight loads hidden and the p-state high).
    """
    EC = E // 128  # e-chunks (contraction over E)
    ST = S // 128  # s-tiles == k-tiles
    QB = S // 512  # query blocks (also projection blocks)
    NSLOT = nslot  # score slots (one head x k-tile each) per PSUM tile
    LAGS = NSLOT * lag  # PV lag in slots

    nc = bacc.Bacc("TRN2", target_bir_lowering=False, debug=False)

    xT_d = nc.dram_tensor("xT", [E, S], F16, kind="ExternalInput")
    wqc_d = nc.dram_tensor("wqc_cols", [E, PAIR], F16, kind="ExternalInput")
    wkc_d = nc.dram_tensor("wkc_cols", [E, PAIR], F16, kind="ExternalInput")
    wvT_d = nc.dram_tensor("wvT_cols", [E, PAIR], F16, kind="ExternalInput")
    out_d = nc.dram_tensor("out", [S, PAIR], F16, kind="ExternalOutput")

    with tile.TileContext(nc) as tc:
        with tc.tile_pool(name="persist", bufs=1) as PST:
            xT_sb = PST.tile([128, EC, S], F16)
            # per-block projection outputs so the main loop can start as
            # soon as block 0 is ready; remaining blocks interleave into
            # the loop's spare PE cycles.
            kTb = [PST.tile([128, 512], F16, name=f"kT_{b}") for b in range(QB)]
            qTb = [PST.tile([128, 512], F16, name=f"qT_{b}") for b in range(QB)]
            # V' per k-chunk: [V_h0 (64) | 1 | V_h1 (64) | 1]
            vb = [PST.tile([128, 4, 130], F16, name=f"v_{b}") for b in range(QB)]
            out_sb = PST.tile([128, ST, PAIR], F16)
            ident = PST.tile([128, 128], F32)
            wqc_sb = PST.tile([128, EC, PAIR], F16)
            wkc_sb = PST.tile([128, EC, PAIR], F16)
            wvT_sb = PST.tile([128, EC, PAIR], F16)
            make_identity(nc, ident[:])
            # memset can't target fp16; stage in fp32 and convert-copy
            ones_f32 = PST.tile([128, 4], F32)
            nc.vector.memset(ones_f32[:], 1.0)

            # ------------- weight + x DMAs (weights host-combined) -------
            for c in range(EC):
                sl = slice(128 * c, 128 * (c + 1))
                nc.sync.dma_start(out=wkc_sb[:, c, :], in_=wkc_d[sl, :])
                nc.sync.dma_start(out=wqc_sb[:, c, :], in_=wqc_d[sl, :])
                nc.sync.dma_start(out=wvT_sb[:, c, :], in_=wvT_d[sl, :])
            for b in range(QB):
                bs = slice(512 * b, 512 * (b + 1))
                for c in range(EC):
                    sl = slice(128 * c, 128 * (c + 1))
                    nc.sync.dma_start(out=xT_sb[:, c, bs], in_=xT_d[sl, bs])

            # ---------------- attention main loop -----------------------
            with (
                tc.tile_pool(name="sc_ps", bufs=sc_bufs, space="PSUM") as SC,
                tc.tile_pool(name="pv_ps", bufs=2, space="PSUM") as PVP,
                tc.tile_pool(name="exp_sb", bufs=11) as EX,
                tc.tile_pool(name="nrm_sb", bufs=2) as NRM,
            ):
                # projection emitters; psum borrowed from the score pool so
                # they can interleave with the loop without extra banks
                def emit_kqT(b, wc, dst, kind):
                    ps = SC.tile([128, NSLOT, 512], F32, tag="sc", name=f"{kind}ps_{b}")
                    bs = slice(512 * b, 512 * (b + 1))
                    for c in range(EC):
                        nc.tensor.matmul(
                            ps[:, 0, :],
                            lhsT=wc[:, c, :],
                            rhs=xT_sb[:, c, bs],
                            start=(c == 0),
                            stop=(c == EC - 1),
                        )
                    nc.vector.tensor_copy(dst[:], ps[:, 0, :])

                def emit_v(b):
                    ps = SC.tile([128, NSLOT, 512], F32, tag="sc", name=f"vps_{b}")
                    view = ps[:, 0, :].rearrange("p (j n) -> p j n", j=4)
                    for jj in range(4):
                        j = 4 * b + jj
                        for c in range(EC):
                            nc.tensor.matmul(
                                view[:, jj, :],
                                lhsT=xT_sb[:, c, 128 * j : 128 * (j + 1)],
                                rhs=wvT_sb[:, c, :],
                                start=(c == 0),
                                stop=(c == EC - 1),
                            )
                    nc.vector.tensor_copy(vb[b][:, :, 0:64], view[:, :, 0:64])
                    nc.vector.tensor_copy(vb[b][:, :, 65:129], view[:, :, 64:128])
                    nc.vector.tensor_copy(vb[b][:, :, 64:65], ones_f32[:])
                    nc.vector.tensor_copy(vb[b][:, :, 129:130], ones_f32[:])

                # minimal pre-loop: block 0 (+1 block of lookahead)
                emit_kqT(0, wkc_sb, kTb[0], "k")
                emit_kqT(0, wqc_sb, qTb[0], "q")
                emit_v(0)
                if QB > 1:
                    emit_kqT(1, wkc_sb, kTb[1], "k")
                    emit_v(1)

                # remaining blocks fill qb0's spare PE cycles: one unit
                # every 2 k-tiles, always ahead of first use (kt = 4b)
                proj_sched = {}
                for b in range(2, QB):
                    proj_sched.setdefault(4 * b - 2, []).append(("k", b))
                    proj_sched.setdefault(4 * b - 1, []).append(("v", b))

                # cyclic exp-engine assignment per slot-group
                group_count = [0]

                def exp_emit(et, sc, n_slots):
                    g = group_count[0]
                    group_count[0] += 1
                    use_dve = dve_pattern[g % len(dve_pattern)]
                    if use_dve:
                        nc.vector.tensor_scalar(
                            et[:, :n_slots, :].bitcast(I16),
                            sc[:, :n_slots, :],
                            SCH_A,
                            SCH_B,
                            MULT,
                            ADD,
                        )
                    else:
                        nc.scalar.activation(
                            et[:, :n_slots, :], sc[:, :n_slots, :], EXPF, scale=0.125
                        )

                # normalize/output for a finished q-block, split in two
                # phases so they can be deferred into the next q-block's
                # score stream without ever stalling the PE.
                def finish_a(fin):
                    qbp, pv, tr, pvS = fin[:4]
                    for h in range(2):
                        pvS.append(
                            NRM.tile([65, 512], F32, tag="pvS", name=f"pvS_{qbp}_{h}")
                        )
                        nc.vector.tensor_copy(pvS[h][:], pv[h][0:65, :])

                def finish_b(fin):
                    qbp, pv, tr, pvS = fin[:4]
                    recs = []
                    for h in range(2):
                        for c4 in range(4):
                            nc.tensor.transpose(
                                tr[h][:, c4, :],
                                pvS[h][:, 128 * c4 : 128 * (c4 + 1)],
                                ident[0:65, 0:65],
                            )
                        rec = NRM.tile([128, 4], F32, tag="rec", name=f"rec_{qbp}_{h}")
                        nc.vector.reciprocal(rec[:], tr[h][:, :, 64])
                        recs.append(rec)
                    COPYF = mybir.ActivationFunctionType.Copy
                    for c4 in range(4):
                        nc.scalar.activation(
                            out_sb[:, 4 * qbp + c4, 0:64],
                            tr[0][:, c4, 0:64],
                            COPYF,
                            scale=recs[0][:, c4 : c4 + 1],
                        )
                    for c4 in range(4):
                        j = 4 * qbp + c4
                        nc.vector.tensor_scalar_mul(
                            out_sb[:, j, 64:128],
                            tr[1][:, c4, 0:64],
                            recs[1][:, c4 : c4 + 1],
                        )
                        nc.sync.dma_start(
                            out=out_d[128 * j : 128 * (j + 1), :],
                            in_=out_sb[:, j, :],
                        )

                pending = None
                for qb in range(QB):
                    pv = [
                        PVP.tile([128, 512], F32, tag="pv", name=f"pv_h0_{qb}"),
                        PVP.tile([128, 512], F32, tag="pv", name=f"pv_h1_{qb}"),
                    ]
                    tr = [
                        PVP.tile([128, 4, 65], F32, tag="pv", name=f"tr_{qb}_{h}")
                        for h in range(2)
                    ]
                    slot_et = [None] * (2 * ST)  # slot -> (exp tile, pos)
                    state = {"sc": None, "et": None, "acted": -1, "pv_next": 0}

                    def emit_pv(s, pv=pv, slot_et=slot_et):
                        kt, h = divmod(s, 2)
                        et, pos = slot_et[s]
                        nc.tensor.matmul(
                            pv[h][0:65, :],
                            lhsT=vb[kt // 4][:, kt % 4, 65 * h : 65 * h + 65],
                            rhs=et[:, pos, :],
                            start=(kt == 0),
                            stop=(kt == ST - 1),
                        )

                    def drain_pv(upto, state=state, emit=emit_pv):
                        while state["pv_next"] <= upto:
                            emit(state["pv_next"])
                            state["pv_next"] += 1

                    for kt in range(ST):
                        if pending is not None:
                            if kt == 2:
                                pending[4]()  # tail PV drain of previous qb
                            elif kt == 3:
                                finish_a(pending)
                            elif kt == 4:
                                finish_b(pending)
                                pending = None
                        if qb == 0:
                            for kind, b in proj_sched.get(kt, ()):
                                if kind == "k":
                                    emit_kqT(b, wkc_sb, kTb[b], "k")
                                else:
                                    emit_v(b)
                        if kt == 16 and qb + 1 < QB:
                            emit_kqT(qb + 1, wqc_sb, qTb[qb + 1], "q")
                        for h in range(2):
                            s = 2 * kt + h
                            pos = s % NSLOT
                            if pos == 0:
                                state["sc"] = SC.tile(
                                    [128, NSLOT, 512], F32, tag="sc", name=f"sc_{qb}_{s}"
                                )
                                state["et"] = EX.tile(
                                    [128, NSLOT, 512], F16, tag="et", name=f"et_{qb}_{s}"
                                )
                            nc.tensor.matmul(
                                state["sc"][:, pos, :],
                                lhsT=kTb[kt // 4][64 * h : 64 * (h + 1), 128 * (kt % 4) : 128 * (kt % 4 + 1)],
                                rhs=qTb[qb][64 * h : 64 * (h + 1), :],
                                start=True,
                                stop=True,
                            )
                            slot_et[s] = (state["et"], pos)
                            if pos == NSLOT - 1:
                                exp_emit(state["et"], state["sc"], NSLOT)
                                state["acted"] = s
                                state["groups"] = state.get("groups", 0) + 1
                                # the first drain of a deferred-finish block
                                # must come after finish_b releases the pv
                                # buffers (else PE deadlocks on its own
                                # later transposes)
                                min_g = burst if qb == 0 else 2 * burst
                                if (
                                    state["groups"] % burst == 0
                                    and state["groups"] >= min_g
                                ):
                                    drain_pv(state["acted"] - LAGS)
                    # flush partial tile; tail PVs are deferred into the
                    # next q-block's score stream (their exps need ~1us)
                    last = 2 * ST - 1
                    if state["acted"] < last:
                        pos = last % NSLOT
                        exp_emit(state["et"], state["sc"], pos + 1)
                    drain_pv(last - 2 * NSLOT)
                    pending = (qb, pv, tr, [], lambda d=drain_pv: d(last), [])

                pending[4]()
                finish_a(pending)
                finish_b(pending)

    nc.compile()
    return nc


_NC_CACHE = {}

BUILD_OPTS = {"lag": 2, "dve_pattern": (0, 0, 1, 0, 1), "nslot": 2,
              "sc_bufs": 3, "burst": 3}


def _get_nc(S=4096):
    key = (S,) + tuple(sorted((k, tuple(v) if isinstance(v, tuple) else v)
                              for k, v in BUILD_OPTS.items()))
    if key not in _NC_CACHE:
        _NC_CACHE[key] = build_attention_nc(S=S, **BUILD_OPTS)
    return _NC_CACHE[key]


def _make_in_maps(rotation_params, entangle_params, inputs, w_q, w_k, w_v):
    B, S, E_ = inputs.shape
    assert E_ == E and B * 4 == N_CORES
    f16 = lambda a: np.ascontiguousarray(np.asarray(a, dtype=np.float16))
    xTs = [f16(np.asarray(inputs[b]).T) for b in range(B)]
    w_q = np.asarray(w_q, dtype=np.float32)
    w_k = np.asarray(w_k, dtype=np.float32)
    rotation_params = np.asarray(rotation_params, dtype=np.float32)
    entangle_params = np.asarray(entangle_params, dtype=np.float32)
    w_v = np.asarray(w_v)
    # combined projection weights (host, fp32 accumulate -> fp16)
    wqc = w_q.T @ rotation_params
    wkc = w_k.T @ entangle_params
    in_maps = []
    for core in range(N_CORES):
        b, m = divmod(core, 4)
        cols = slice(PAIR * m, PAIR * (m + 1))
        in_maps.append(
            {
                "xT": xTs[b],
                "wqc_cols": f16(wqc[:, cols]),
                "wkc_cols": f16(wkc[:, cols]),
                "wvT_cols": f16(w_v[cols, :].T),
            }
        )
    return in_maps


def run(rotation_params, entangle_params, inputs, w_q, w_k, w_v, trace=False):
    """Run on the 8 NeuronCores; returns (output, BassKernelResults)."""
    inputs = np.asarray(inputs)
    B, S, E_ = inputs.shape
    nc = _get_nc(S)
    in_maps = _make_in_maps(rotation_params, entangle_params, inputs, w_q, w_k, w_v)
    res = run_bass_kernel_spmd(nc, in_maps, list(range(N_CORES)), trace=trace)
    out = np.empty((B, S, E_), dtype=np.float32)
    for core in range(N_CORES):
        b, m = divmod(core, 4)
        out[b, :, PAIR * m : PAIR * (m + 1)] = np.asarray(
            res.results[core]["out"], dtype=np.float32
        )
    return out, res


def kernel(rotation_params, entangle_params, inputs, w_q, w_k, w_v):
    out, _ = run(rotation_params, entangle_params, inputs, w_q, w_k, w_v)
    return out


# revision 29
# speedup vs baseline: 1.1487x; 1.0510x over previous
"""Trainium2 Bass kernel for ClassicalSelfAttention.

Math (per batch b):
    q = (x @ w_q.T) @ R ; k = (x @ w_k.T) @ Ent ; v = x @ w_v.T
    per head h: out_h = softmax(q_h @ k_h.T / sqrt(64)) @ v_h
    out[b, s, h*64+d]

Sharding: 8 cores, core i handles batch b = i // 4 and the adjacent head
pair m = i % 4 (global heads 2m, 2m+1 -> output columns 128m..128m+128).
Weights are replicated (column/row-sliced per head pair on the host); no
inter-core communication.

Per-core device plan (S = 4096, E = 512, D = 64, 2 heads):
  - host passes x^T and all weights in fp16 (halves input DMA and removes
    the on-device fp32->fp16 conversion copies; rel error of the fp16
    pipeline vs fp32 reference is ~7e-4).
  - combined projection weights Wq_comb = w_q.T @ rot_cols (and w_k/ent)
    are computed on PE in fp16 (1 cyc/row; fp32r had a 4x penalty at
    free=128), so q/k projections are single matmuls per 128-chunk.
  - Q^T, K^T stacked [128 = 2 heads x 64, S] in SBUF; V' = [V | ones]
    per head ([S, 65]) so the softmax denominator falls out of the PV
    matmul's extra column.
  - scores^T[k, q] = (K^T tile).T @ Q^T per head; the PE stream cost is
    the output free size (512 q per instruction), so scores+PV together
    are PSUM-write-port bound at ~221us/core; everything else must hide
    under that.
  - exp is split across two engines (ScalarE alone would be ~254us):
      * ACT: nc.scalar.activation(Exp, scale=1/8) PSUM->SBUF fp16.
      * DVE: one tensor_scalar per tile group computing the Schraudolph
        bit-trick in the fp16 domain: int16(s * 1024*log2(e)/8 + B)
        bit-cast as fp16 IS approximately exp(s/8) (max rel err ~4%,
        which softmax normalization mostly cancels; measured end-to-end
        rel-l2 ~5e-3 at a 1/3 DVE share).
    Groups are assigned round-robin ACT:DVE = 2:1.
  - PV: out'^T[65, q] = V'.T @ exp_sT accumulated over the 32 k-chunks in
    one PSUM bank per head; row 64 is the denominator. PE-transpose to
    natural layout, scale by reciprocal denominator, per-q-block DMA out.
  - Projections are streamed into the first q-block's spare PE cycles so
    the exp stream starts early.
"""

import sys

if "/opt/trn_rl_repo" not in sys.path:
    sys.path.insert(0, "/opt/trn_rl_repo")

import numpy as np

import concourse.bass as bass  # noqa: F401  (engine namespaces live on nc)
import concourse.mybir as mybir
import concourse.tile as tile
from concourse import bacc
from concourse.bass_utils import run_bass_kernel_spmd
from concourse.masks import make_identity

F32 = mybir.dt.float32
F16 = mybir.dt.float16
I16 = mybir.dt.int16
EXPF = mybir.ActivationFunctionType.Exp
MULT = mybir.AluOpType.mult
ADD = mybir.AluOpType.add

E = 512
D = 64
PAIR = 128  # 2 heads x 64 dims per core
N_CORES = 8

# Schraudolph constants for exp(s/8) in fp16-bit domain:
#   fp16_bits(exp(s/8)) ~= s * (1024*log2(e)/8) + (15*1024 - C)
SCH_A = float(np.float32(1024.0 * np.log2(np.e) / 8.0))
SCH_B = float(np.float32(15360.0 - 60.0))


def build_attention_nc(S=4096, lag=2, dve_pattern=(0, 0, 1, 0, 1), nslot=2,
                       sc_bufs=3, burst=3):
    """Build the single-core Bass program (SPMD: every core runs this).

    dve_pattern: cyclic pattern over exp slot-groups; 1 -> DVE Schraudolph,
    0 -> ACT exp.  nslot: score tiles per PSUM group.  sc_bufs: rotating
    PSUM group buffers.  burst: PV emission batching in groups (longer
    same-kind PE runs keep weight loads hidden and the p-state high).
    """
    EC = E // 128  # e-chunks (contraction over E)
    ST = S // 128  # s-tiles == k-tiles
    QB = S // 512  # query blocks (also projection blocks)
    NSLOT = nslot  # score slots (one head x k-tile each) per PSUM tile
    LAGS = NSLOT * lag  # PV lag in slots

    nc = bacc.Bacc("TRN2", target_bir_lowering=False, debug=False)

    xT_d = nc.dram_tensor("xT", [E, S], F16, kind="ExternalInput")
    wqc_d = nc.dram_tensor("wqc_cols", [E, PAIR], F16, kind="ExternalInput")
    wkc_d = nc.dram_tensor("wkc_cols", [E, PAIR], F16, kind="ExternalInput")
    wvT_d = nc.dram_tensor("wvT_cols", [E, PAIR], F16, kind="ExternalInput")
    out_d = nc.dram_tensor("out", [S, PAIR], F16, kind="ExternalOutput")

    with tile.TileContext(nc) as tc:
        with tc.tile_pool(name="persist", bufs=1) as PST:
            xT_sb = PST.tile([128, EC, S], F16)
            # per-block projection outputs so the main loop can start as
            # soon as block 0 is ready; remaining blocks interleave into
            # the loop's spare PE cycles.
            kTb = [PST.tile([128, 512], F16, name=f"kT_{b}") for b in range(QB)]
            qTb = [PST.tile([128, 512], F16, name=f"qT_{b}") for b in range(QB)]
            # V' per k-chunk: [V_h0 (64) | 1 | V_h1 (64) | 1]
            vb = [PST.tile([128, 4, 130], F16, name=f"v_{b}") for b in range(QB)]
            out_sb = PST.tile([128, ST, PAIR], F16)
            ident = PST.tile([128, 128], F32)
            wqc_sb = PST.tile([128, EC, PAIR], F16)
            wkc_sb = PST.tile([128, EC, PAIR], F16)
            wvT_sb = PST.tile([128, EC, PAIR], F16)
            make_identity(nc, ident[:])
            # memset can't target fp16; stage in fp32 and convert-copy
            ones_f32 = PST.tile([128, 4], F32)
            nc.vector.memset(ones_f32[:], 1.0)

            # ------------- weight + x DMAs (weights host-combined) -------
            for c in range(EC):
                sl = slice(128 * c, 128 * (c + 1))
                nc.sync.dma_start(out=wkc_sb[:, c, :], in_=wkc_d[sl, :])
                nc.sync.dma_start(out=wqc_sb[:, c, :], in_=wqc_d[sl, :])
                nc.sync.dma_start(out=wvT_sb[:, c, :], in_=wvT_d[sl, :])
            for b in range(QB):
                bs = slice(512 * b, 512 * (b + 1))
                for c in range(EC):
                    sl = slice(128 * c, 128 * (c + 1))
                    nc.sync.dma_start(out=xT_sb[:, c, bs], in_=xT_d[sl, bs])

            # ---------------- attention main loop -----------------------
            with (
                tc.tile_pool(name="sc_ps", bufs=sc_bufs, space="PSUM") as SC,
                tc.tile_pool(name="pv_ps", bufs=2, space="PSUM") as PVP,
                tc.tile_pool(name="exp_sb", bufs=11) as EX,
                tc.tile_pool(name="nrm_sb", bufs=2) as NRM,
            ):
                # projection emitters; psum borrowed from the score pool so
                # they can interleave with the loop without extra banks
                def emit_kqT(b, wc, dst, kind):
                    ps = SC.tile([128, NSLOT, 512], F32, tag="sc", name=f"{kind}ps_{b}")
                    bs = slice(512 * b, 512 * (b + 1))
                    for c in range(EC):
                        nc.tensor.matmul(
                            ps[:, 0, :],
                            lhsT=wc[:, c, :],
                            rhs=xT_sb[:, c, bs],
                            start=(c == 0),
                            stop=(c == EC - 1),
                        )
                    nc.vector.tensor_copy(dst[:], ps[:, 0, :])

                def emit_v(b):
                    ps = SC.tile([128, NSLOT, 512], F32, tag="sc", name=f"vps_{b}")
                    view = ps[:, 0, :].rearrange("p (j n) -> p j n", j=4)
                    for jj in range(4):
                        j = 4 * b + jj
                        for c in range(EC):
                            nc.tensor.matmul(
                                view[:, jj, :],
                                lhsT=xT_sb[:, c, 128 * j : 128 * (j + 1)],
                                rhs=wvT_sb[:, c, :],
                                start=(c == 0),
                                stop=(c == EC - 1),
                            )
                    nc.vector.tensor_copy(vb[b][:, :, 0:64], view[:, :, 0:64])
                    nc.vector.tensor_copy(vb[b][:, :, 65:129], view[:, :, 64:128])
                    nc.vector.tensor_copy(vb[b][:, :, 64:65], ones_f32[:])
                    nc.vector.tensor_copy(vb[b][:, :, 129:130], ones_f32[:])

                # minimal pre-loop: block 0 (+1 block of lookahead)
                emit_kqT(0, wkc_sb, kTb[0], "k")
                emit_kqT(0, wqc_sb, qTb[0], "q")
                emit_v(0)
                if QB > 1:
                    emit_kqT(1, wkc_sb, kTb[1], "k")
                    emit_v(1)

                # remaining blocks fill qb0's spare PE cycles: one unit
                # every 2 k-tiles, always ahead of first use (kt = 4b)
                proj_sched = {}
                for b in range(2, QB):
                    proj_sched.setdefault(4 * b - 2, []).append(("k", b))
                    proj_sched.setdefault(4 * b - 1, []).append(("v", b))

                # cyclic exp-engine assignment per slot-group
                group_count = [0]

                def exp_emit(et, sc, n_slots):
                    g = group_count[0]
                    group_count[0] += 1
                    use_dve = dve_pattern[g % len(dve_pattern)]
                    if use_dve:
                        nc.vector.tensor_scalar(
                            et[:, :n_slots, :].bitcast(I16),
                            sc[:, :n_slots, :],
                            SCH_A,
                            SCH_B,
                            MULT,
                            ADD,
                        )
                    else:
                        nc.scalar.activation(
                            et[:, :n_slots, :], sc[:, :n_slots, :], EXPF, scale=0.125
                        )

                # normalize/output for a finished q-block, split in two
                # phases so they can be deferred into the next q-block's
                # score stream without ever stalling the PE.
                def finish_a(fin):
                    qbp, pv, tr, pvS = fin[:4]
                    for h in range(2):
                        pvS.append(
                            NRM.tile([65, 512], F32, tag="pvS", name=f"pvS_{qbp}_{h}")
                        )
                        nc.vector.tensor_copy(pvS[h][:], pv[h][0:65, :])

                def finish_b(fin):
                    qbp, pv, tr, pvS = fin[:4]
                    recs = []
                    for h in range(2):
                        for c4 in range(4):
                            nc.tensor.transpose(
                                tr[h][:, c4, :],
                                pvS[h][:, 128 * c4 : 128 * (c4 + 1)],
                                ident[0:65, 0:65],
                            )
                        rec = NRM.tile([128, 4], F32, tag="rec", name=f"rec_{qbp}_{h}")
                        nc.vector.reciprocal(rec[:], tr[h][:, :, 64])
                        recs.append(rec)
                    for c4 in range(4):
                        j = 4 * qbp + c4
                        for h in range(2):
                            nc.vector.tensor_scalar_mul(
                                out_sb[:, j, 64 * h : 64 * (h + 1)],
                                tr[h][:, c4, 0:64],
                                recs[h][:, c4 : c4 + 1],
                            )
                        nc.sync.dma_start(
                            out=out_d[128 * j : 128 * (j + 1), :],
                            in_=out_sb[:, j, :],
                        )

                pending = None
                for qb in range(QB):
                    pv = [
                        PVP.tile([128, 512], F32, tag="pv", name=f"pv_h0_{qb}"),
                        PVP.tile([128, 512], F32, tag="pv", name=f"pv_h1_{qb}"),
                    ]
                    tr = [
                        PVP.tile([128, 4, 65], F32, tag="pv", name=f"tr_{qb}_{h}")
                        for h in range(2)
                    ]
                    slot_et = [None] * (2 * ST)  # slot -> (exp tile, pos)
                    state = {"sc": None, "et": None, "acted": -1, "pv_next": 0}

                    def emit_pv(s, pv=pv, slot_et=slot_et):
                        kt, h = divmod(s, 2)
                        et, pos = slot_et[s]
                        nc.tensor.matmul(
                            pv[h][0:65, :],
                            lhsT=vb[kt // 4][:, kt % 4, 65 * h : 65 * h + 65],
                            rhs=et[:, pos, :],
                            start=(kt == 0),
                            stop=(kt == ST - 1),
                        )

                    def drain_pv(upto, state=state, emit=emit_pv):
                        while state["pv_next"] <= upto:
                            emit(state["pv_next"])
                            state["pv_next"] += 1

                    for kt in range(ST):
                        if pending is not None:
                            if kt == 2:
                                pending[4]()  # tail PV drain of previous qb
                            elif kt == 3:
                                finish_a(pending)
                            elif kt == 4:
                                finish_b(pending)
                                pending = None
                        if qb == 0:
                            for kind, b in proj_sched.get(kt, ()):
                                if kind == "k":
                                    emit_kqT(b, wkc_sb, kTb[b], "k")
                                else:
                                    emit_v(b)
                        if kt == 16 and qb + 1 < QB:
                            emit_kqT(qb + 1, wqc_sb, qTb[qb + 1], "q")
                        for h in range(2):
                            s = 2 * kt + h
                            pos = s % NSLOT
                            if pos == 0:
                                state["sc"] = SC.tile(
                                    [128, NSLOT, 512], F32, tag="sc", name=f"sc_{qb}_{s}"
                                )
                                state["et"] = EX.tile(
                                    [128, NSLOT, 512], F16, tag="et", name=f"et_{qb}_{s}"
                                )
                            nc.tensor.matmul(
                                state["sc"][:, pos, :],
                                lhsT=kTb[kt // 4][64 * h : 64 * (h + 1), 128 * (kt % 4) : 128 * (kt % 4 + 1)],
                                rhs=qTb[qb][64 * h : 64 * (h + 1), :],
                                start=True,
                                stop=True,
                            )
                            slot_et[s] = (state["et"], pos)
                            if pos == NSLOT - 1:
                                exp_emit(state["et"], state["sc"], NSLOT)
                                state["acted"] = s
                                state["groups"] = state.get("groups", 0) + 1
                                # the first drain of a deferred-finish block
                                # must come after finish_b releases the pv
                                # buffers (else PE deadlocks on its own
                                # later transposes)
                                min_g = burst if qb == 0 else 2 * burst
                                if (
                                    state["groups"] % burst == 0
                                    and state["groups"] >= min_g
                                ):
                                    drain_pv(state["acted"] - LAGS)
                    # flush partial tile; tail PVs are deferred into the
                    # next q-block's score stream (their exps need ~1us)
                    last = 2 * ST - 1
                    if state["acted"] < last:
                        pos = last % NSLOT
                        exp_emit(state["et"], state["sc"], pos + 1)
                    drain_pv(last - 2 * NSLOT)
                    pending = (qb, pv, tr, [], lambda d=drain_pv: d(last), [])

                pending[4]()
                finish_a(pending)
                finish_b(pending)

    nc.compile()
    return nc


_NC_CACHE = {}

BUILD_OPTS = {"lag": 2, "dve_pattern": (0, 0, 1, 0, 1), "nslot": 2,
              "sc_bufs": 3, "burst": 3}


def _get_nc(S=4096):
    key = (S,) + tuple(sorted((k, tuple(v) if isinstance(v, tuple) else v)
                              for k, v in BUILD_OPTS.items()))
    if key not in _NC_CACHE:
        _NC_CACHE[key] = build_attention_nc(S=S, **BUILD_OPTS)
    return _NC_CACHE[key]


def _make_in_maps(rotation_params, entangle_params, inputs, w_q, w_k, w_v):
    B, S, E_ = inputs.shape
    assert E_ == E and B * 4 == N_CORES
    f16 = lambda a: np.ascontiguousarray(np.asarray(a, dtype=np.float16))
    xTs = [f16(np.asarray(inputs[b]).T) for b in range(B)]
    w_q = np.asarray(w_q, dtype=np.float32)
    w_k = np.asarray(w_k, dtype=np.float32)
    rotation_params = np.asarray(rotation_params, dtype=np.float32)
    entangle_params = np.asarray(entangle_params, dtype=np.float32)
    w_v = np.asarray(w_v)
    # combined projection weights (host, fp32 accumulate -> fp16)
    wqc = w_q.T @ rotation_params
    wkc = w_k.T @ entangle_params
    in_maps = []
    for core in range(N_CORES):
        b, m = divmod(core, 4)
        cols = slice(PAIR * m, PAIR * (m + 1))
        in_maps.append(
            {
                "xT": xTs[b],
                "wqc_cols": f16(wqc[:, cols]),
                "wkc_cols": f16(wkc[:, cols]),
                "wvT_cols": f16(w_v[cols, :].T),
            }
        )
    return in_maps


def run(rotation_params, entangle_params, inputs, w_q, w_k, w_v, trace=False):
    """Run on the 8 NeuronCores; returns (output, BassKernelResults)."""
    inputs = np.asarray(inputs)
    B, S, E_ = inputs.shape
    nc = _get_nc(S)
    in_maps = _make_in_maps(rotation_params, entangle_params, inputs, w_q, w_k, w_v)
    res = run_bass_kernel_spmd(nc, in_maps, list(range(N_CORES)), trace=trace)
    out = np.empty((B, S, E_), dtype=np.float32)
    for core in range(N_CORES):
        b, m = divmod(core, 4)
        out[b, :, PAIR * m : PAIR * (m + 1)] = np.asarray(
            res.results[core]["out"], dtype=np.float32
        )
    return out, res


def kernel(rotation_params, entangle_params, inputs, w_q, w_k, w_v):
    out, _ = run(rotation_params, entangle_params, inputs, w_q, w_k, w_v)
    return out


# revision 30
# speedup vs baseline: 1.1571x; 1.0073x over previous
"""Trainium2 Bass kernel for ClassicalSelfAttention.

Math (per batch b):
    q = (x @ w_q.T) @ R ; k = (x @ w_k.T) @ Ent ; v = x @ w_v.T
    per head h: out_h = softmax(q_h @ k_h.T / sqrt(64)) @ v_h
    out[b, s, h*64+d]

Sharding: 8 cores, core i handles batch b = i // 4 and the adjacent head
pair m = i % 4 (global heads 2m, 2m+1 -> output columns 128m..128m+128).
Weights are replicated (column/row-sliced per head pair on the host); no
inter-core communication.

Per-core device plan (S = 4096, E = 512, D = 64, 2 heads):
  - host passes x^T and all weights in fp16 (halves input DMA and removes
    the on-device fp32->fp16 conversion copies; rel error of the fp16
    pipeline vs fp32 reference is ~7e-4).
  - combined projection weights Wq_comb = w_q.T @ rot_cols (and w_k/ent)
    are computed on PE in fp16 (1 cyc/row; fp32r had a 4x penalty at
    free=128), so q/k projections are single matmuls per 128-chunk.
  - Q^T, K^T stacked [128 = 2 heads x 64, S] in SBUF; V' = [V | ones]
    per head ([S, 65]) so the softmax denominator falls out of the PV
    matmul's extra column.
  - scores^T[k, q] = (K^T tile).T @ Q^T per head; the PE stream cost is
    the output free size (512 q per instruction), so scores+PV together
    are PSUM-write-port bound at ~221us/core; everything else must hide
    under that.
  - exp is split across two engines (ScalarE alone would be ~254us):
      * ACT: nc.scalar.activation(Exp, scale=1/8) PSUM->SBUF fp16.
      * DVE: one tensor_scalar per tile group computing the Schraudolph
        bit-trick in the fp16 domain: int16(s * 1024*log2(e)/8 + B)
        bit-cast as fp16 IS approximately exp(s/8) (max rel err ~4%,
        which softmax normalization mostly cancels; measured end-to-end
        rel-l2 ~5e-3 at a 1/3 DVE share).
    Groups are assigned round-robin ACT:DVE = 2:1.
  - PV: out'^T[65, q] = V'.T @ exp_sT accumulated over the 32 k-chunks in
    one PSUM bank per head; row 64 is the denominator. PE-transpose to
    natural layout, scale by reciprocal denominator, per-q-block DMA out.
  - Projections are streamed into the first q-block's spare PE cycles so
    the exp stream starts early.
"""

import sys

if "/opt/trn_rl_repo" not in sys.path:
    sys.path.insert(0, "/opt/trn_rl_repo")

import numpy as np

import concourse.bass as bass  # noqa: F401  (engine namespaces live on nc)
import concourse.mybir as mybir
import concourse.tile as tile
from concourse import bacc
from concourse.bass_utils import run_bass_kernel_spmd
from concourse.masks import make_identity

F32 = mybir.dt.float32
F16 = mybir.dt.float16
I16 = mybir.dt.int16
EXPF = mybir.ActivationFunctionType.Exp
MULT = mybir.AluOpType.mult
ADD = mybir.AluOpType.add

E = 512
D = 64
PAIR = 128  # 2 heads x 64 dims per core
N_CORES = 8

# Schraudolph constants for exp(s/8) in fp16-bit domain:
#   fp16_bits(exp(s/8)) ~= s * (1024*log2(e)/8) + (15*1024 - C)
SCH_A = float(np.float32(1024.0 * np.log2(np.e) / 8.0))
SCH_B = float(np.float32(15360.0 - 60.0))


def build_attention_nc(S=4096, lag=2, dve_pattern=(0, 0, 1, 0, 1), nslot=2,
                       sc_bufs=3, burst=3):
    """Build the single-core Bass program (SPMD: every core runs this).

    dve_pattern: cyclic pattern over exp slot-groups; 1 -> DVE Schraudolph,
    0 -> ACT exp.  nslot: score tiles per PSUM group.  sc_bufs: rotating
    PSUM group buffers.  burst: PV emission batching in groups (longer
    same-kind PE runs keep weight loads hidden and the p-state high).
    """
    EC = E // 128  # e-chunks (contraction over E)
    ST = S // 128  # s-tiles == k-tiles
    QB = S // 512  # query blocks (also projection blocks)
    NSLOT = nslot  # score slots (one head x k-tile each) per PSUM tile
    LAGS = NSLOT * lag  # PV lag in slots

    nc = bacc.Bacc("TRN2", target_bir_lowering=False, debug=False)

    xT_d = nc.dram_tensor("xT", [E, S], F16, kind="ExternalInput")
    wqc_d = nc.dram_tensor("wqc_cols", [E, PAIR], F16, kind="ExternalInput")
    wkc_d = nc.dram_tensor("wkc_cols", [E, PAIR], F16, kind="ExternalInput")
    wvT_d = nc.dram_tensor("wvT_cols", [E, PAIR], F16, kind="ExternalInput")
    out_d = nc.dram_tensor("out", [S, PAIR], F16, kind="ExternalOutput")

    with tile.TileContext(nc) as tc:
        with tc.tile_pool(name="persist", bufs=1) as PST:
            xT_sb = PST.tile([128, EC, S], F16)
            # per-block projection outputs so the main loop can start as
            # soon as block 0 is ready; remaining blocks interleave into
            # the loop's spare PE cycles.
            kTb = [PST.tile([128, 512], F16, name=f"kT_{b}") for b in range(QB)]
            qTb = [PST.tile([128, 512], F16, name=f"qT_{b}") for b in range(QB)]
            # V' per k-chunk: [V_h0 (64) | 1 | V_h1 (64) | 1]
            vb = [PST.tile([128, 4, 130], F16, name=f"v_{b}") for b in range(QB)]
            out_sb = PST.tile([128, ST, PAIR], F16)
            ident = PST.tile([128, 128], F32)
            wqc_sb = PST.tile([128, EC, PAIR], F16)
            wkc_sb = PST.tile([128, EC, PAIR], F16)
            wvT_sb = PST.tile([128, EC, PAIR], F16)
            make_identity(nc, ident[:])
            # memset can't target fp16; stage in fp32 and convert-copy
            ones_f32 = PST.tile([128, 4], F32)
            nc.vector.memset(ones_f32[:], 1.0)

            # ------------- weight + x DMAs (weights host-combined) -------
            for c in range(EC):
                sl = slice(128 * c, 128 * (c + 1))
                nc.sync.dma_start(out=wkc_sb[:, c, :], in_=wkc_d[sl, :])
                nc.sync.dma_start(out=wqc_sb[:, c, :], in_=wqc_d[sl, :])
                nc.sync.dma_start(out=wvT_sb[:, c, :], in_=wvT_d[sl, :])
            for b in range(QB):
                bs = slice(512 * b, 512 * (b + 1))
                for c in range(EC):
                    sl = slice(128 * c, 128 * (c + 1))
                    nc.sync.dma_start(out=xT_sb[:, c, bs], in_=xT_d[sl, bs])

            # ---------------- attention main loop -----------------------
            with (
                tc.tile_pool(name="sc_ps", bufs=sc_bufs, space="PSUM") as SC,
                tc.tile_pool(name="pv_ps", bufs=2, space="PSUM") as PVP,
                tc.tile_pool(name="exp_sb", bufs=11) as EX,
                tc.tile_pool(name="nrm_sb", bufs=2) as NRM,
            ):
                # projection emitters; psum borrowed from the score pool so
                # they can interleave with the loop without extra banks
                def emit_kqT(b, wc, dst, kind):
                    ps = SC.tile([128, NSLOT, 512], F32, tag="sc", name=f"{kind}ps_{b}")
                    bs = slice(512 * b, 512 * (b + 1))
                    for c in range(EC):
                        nc.tensor.matmul(
                            ps[:, 0, :],
                            lhsT=wc[:, c, :],
                            rhs=xT_sb[:, c, bs],
                            start=(c == 0),
                            stop=(c == EC - 1),
                        )
                    nc.vector.tensor_copy(dst[:], ps[:, 0, :])

                def emit_v(b):
                    ps = SC.tile([128, NSLOT, 512], F32, tag="sc", name=f"vps_{b}")
                    view = ps[:, 0, :].rearrange("p (j n) -> p j n", j=4)
                    for jj in range(4):
                        j = 4 * b + jj
                        for c in range(EC):
                            nc.tensor.matmul(
                                view[:, jj, :],
                                lhsT=xT_sb[:, c, 128 * j : 128 * (j + 1)],
                                rhs=wvT_sb[:, c, :],
                                start=(c == 0),
                                stop=(c == EC - 1),
                            )
                    nc.vector.tensor_copy(vb[b][:, :, 0:64], view[:, :, 0:64])
                    nc.vector.tensor_copy(vb[b][:, :, 65:129], view[:, :, 64:128])
                    nc.vector.tensor_copy(vb[b][:, :, 64:65], ones_f32[:])
                    nc.vector.tensor_copy(vb[b][:, :, 129:130], ones_f32[:])

                # minimal pre-loop: block 0 (+1 block of lookahead)
                emit_kqT(0, wkc_sb, kTb[0], "k")
                emit_kqT(0, wqc_sb, qTb[0], "q")
                emit_v(0)
                if QB > 1:
                    emit_kqT(1, wkc_sb, kTb[1], "k")
                    emit_v(1)

                # remaining blocks fill qb0's spare PE cycles: one unit
                # every 2 k-tiles, always ahead of first use (kt = 4b)
                proj_sched = {}
                for b in range(2, QB):
                    proj_sched.setdefault(4 * b - 2, []).append(("k", b))
                    proj_sched.setdefault(4 * b - 1, []).append(("v", b))

                # cyclic exp-engine assignment per slot-group
                group_count = [0]

                def exp_emit(et, sc, n_slots):
                    g = group_count[0]
                    group_count[0] += 1
                    use_dve = dve_pattern[g % len(dve_pattern)]
                    if use_dve:
                        nc.vector.tensor_scalar(
                            et[:, :n_slots, :].bitcast(I16),
                            sc[:, :n_slots, :],
                            SCH_A,
                            SCH_B,
                            MULT,
                            ADD,
                        )
                    else:
                        nc.scalar.activation(
                            et[:, :n_slots, :], sc[:, :n_slots, :], EXPF, scale=0.125
                        )

                # normalize/output for a finished q-block, split in two
                # phases so they can be deferred into the next q-block's
                # score stream without ever stalling the PE.
                COPYF = mybir.ActivationFunctionType.Copy

                def finish_a(fin):
                    qbp, pv, tr, pvS = fin[:4]
                    for h in range(2):
                        pvS.append(
                            NRM.tile([65, 512], F32, tag="pvS", name=f"pvS_{qbp}_{h}")
                        )
                        if h == 0:
                            nc.scalar.activation(pvS[h][:], pv[h][0:65, :], COPYF)
                        else:
                            nc.vector.tensor_copy(pvS[h][:], pv[h][0:65, :])

                def finish_b(fin):
                    qbp, pv, tr, pvS = fin[:4]
                    recs = []
                    for h in range(2):
                        for c4 in range(4):
                            nc.tensor.transpose(
                                tr[h][:, c4, :],
                                pvS[h][:, 128 * c4 : 128 * (c4 + 1)],
                                ident[0:65, 0:65],
                            )
                        rec = NRM.tile([128, 4], F32, tag="rec", name=f"rec_{qbp}_{h}")
                        nc.vector.reciprocal(rec[:], tr[h][:, :, 64])
                        recs.append(rec)
                    for c4 in range(4):
                        j = 4 * qbp + c4
                        for h in range(2):
                            nc.vector.tensor_scalar_mul(
                                out_sb[:, j, 64 * h : 64 * (h + 1)],
                                tr[h][:, c4, 0:64],
                                recs[h][:, c4 : c4 + 1],
                            )
                        nc.sync.dma_start(
                            out=out_d[128 * j : 128 * (j + 1), :],
                            in_=out_sb[:, j, :],
                        )

                pending = None
                for qb in range(QB):
                    pv = [
                        PVP.tile([128, 512], F32, tag="pv", name=f"pv_h0_{qb}"),
                        PVP.tile([128, 512], F32, tag="pv", name=f"pv_h1_{qb}"),
                    ]
                    tr = [
                        PVP.tile([128, 4, 65], F32, tag="pv", name=f"tr_{qb}_{h}")
                        for h in range(2)
                    ]
                    slot_et = [None] * (2 * ST)  # slot -> (exp tile, pos)
                    state = {"sc": None, "et": None, "acted": -1, "pv_next": 0}

                    def emit_pv(s, pv=pv, slot_et=slot_et):
                        kt, h = divmod(s, 2)
                        et, pos = slot_et[s]
                        nc.tensor.matmul(
                            pv[h][0:65, :],
                            lhsT=vb[kt // 4][:, kt % 4, 65 * h : 65 * h + 65],
                            rhs=et[:, pos, :],
                            start=(kt == 0),
                            stop=(kt == ST - 1),
                        )

                    def drain_pv(upto, state=state, emit=emit_pv):
                        while state["pv_next"] <= upto:
                            emit(state["pv_next"])
                            state["pv_next"] += 1

                    for kt in range(ST):
                        if pending is not None:
                            if kt == 2:
                                pending[4]()  # tail PV drain of previous qb
                            elif kt == 3:
                                finish_a(pending)
                            elif kt == 4:
                                finish_b(pending)
                                pending = None
                        if qb == 0:
                            for kind, b in proj_sched.get(kt, ()):
                                if kind == "k":
                                    emit_kqT(b, wkc_sb, kTb[b], "k")
                                else:
                                    emit_v(b)
                        if kt == 16 and qb + 1 < QB:
                            emit_kqT(qb + 1, wqc_sb, qTb[qb + 1], "q")
                        for h in range(2):
                            s = 2 * kt + h
                            pos = s % NSLOT
                            if pos == 0:
                                state["sc"] = SC.tile(
                                    [128, NSLOT, 512], F32, tag="sc", name=f"sc_{qb}_{s}"
                                )
                                state["et"] = EX.tile(
                                    [128, NSLOT, 512], F16, tag="et", name=f"et_{qb}_{s}"
                                )
                            nc.tensor.matmul(
                                state["sc"][:, pos, :],
                                lhsT=kTb[kt // 4][64 * h : 64 * (h + 1), 128 * (kt % 4) : 128 * (kt % 4 + 1)],
                                rhs=qTb[qb][64 * h : 64 * (h + 1), :],
                                start=True,
                                stop=True,
                            )
                            slot_et[s] = (state["et"], pos)
                            if pos == NSLOT - 1:
                                exp_emit(state["et"], state["sc"], NSLOT)
                                state["acted"] = s
                                state["groups"] = state.get("groups", 0) + 1
                                # the first drain of a deferred-finish block
                                # must come after finish_b releases the pv
                                # buffers (else PE deadlocks on its own
                                # later transposes)
                                min_g = burst if qb == 0 else 2 * burst
                                if (
                                    state["groups"] % burst == 0
                                    and state["groups"] >= min_g
                                ):
                                    drain_pv(state["acted"] - LAGS)
                    # flush partial tile; tail PVs are deferred into the
                    # next q-block's score stream (their exps need ~1us)
                    last = 2 * ST - 1
                    if state["acted"] < last:
                        pos = last % NSLOT
                        exp_emit(state["et"], state["sc"], pos + 1)
                    drain_pv(last - 2 * NSLOT)
                    pending = (qb, pv, tr, [], lambda d=drain_pv: d(last), [])

                pending[4]()
                finish_a(pending)
                finish_b(pending)

    nc.compile()
    return nc


_NC_CACHE = {}

BUILD_OPTS = {"lag": 2, "dve_pattern": (0, 0, 1, 0, 1), "nslot": 2,
              "sc_bufs": 3, "burst": 3}


def _get_nc(S=4096):
    key = (S,) + tuple(sorted((k, tuple(v) if isinstance(v, tuple) else v)
                              for k, v in BUILD_OPTS.items()))
    if key not in _NC_CACHE:
        _NC_CACHE[key] = build_attention_nc(S=S, **BUILD_OPTS)
    return _NC_CACHE[key]


def _make_in_maps(rotation_params, entangle_params, inputs, w_q, w_k, w_v):
    B, S, E_ = inputs.shape
    assert E_ == E and B * 4 == N_CORES
    f16 = lambda a: np.ascontiguousarray(np.asarray(a, dtype=np.float16))
    xTs = [f16(np.asarray(inputs[b]).T) for b in range(B)]
    w_q = np.asarray(w_q, dtype=np.float32)
    w_k = np.asarray(w_k, dtype=np.float32)
    rotation_params = np.asarray(rotation_params, dtype=np.float32)
    entangle_params = np.asarray(entangle_params, dtype=np.float32)
    w_v = np.asarray(w_v)
    # combined projection weights (host, fp32 accumulate -> fp16)
    wqc = w_q.T @ rotation_params
    wkc = w_k.T @ entangle_params
    in_maps = []
    for core in range(N_CORES):
        b, m = divmod(core, 4)
        cols = slice(PAIR * m, PAIR * (m + 1))
        in_maps.append(
            {
                "xT": xTs[b],
                "wqc_cols": f16(wqc[:, cols]),
                "wkc_cols": f16(wkc[:, cols]),
                "wvT_cols": f16(w_v[cols, :].T),
            }
        )
    return in_maps


def run(rotation_params, entangle_params, inputs, w_q, w_k, w_v, trace=False):
    """Run on the 8 NeuronCores; returns (output, BassKernelResults)."""
    inputs = np.asarray(inputs)
    B, S, E_ = inputs.shape
    nc = _get_nc(S)
    in_maps = _make_in_maps(rotation_params, entangle_params, inputs, w_q, w_k, w_v)
    res = run_bass_kernel_spmd(nc, in_maps, list(range(N_CORES)), trace=trace)
    out = np.empty((B, S, E_), dtype=np.float32)
    for core in range(N_CORES):
        b, m = divmod(core, 4)
        out[b, :, PAIR * m : PAIR * (m + 1)] = np.asarray(
            res.results[core]["out"], dtype=np.float32
        )
    return out, res


def kernel(rotation_params, entangle_params, inputs, w_q, w_k, w_v):
    out, _ = run(rotation_params, entangle_params, inputs, w_q, w_k, w_v)
    return out
